# revision 1
# baseline (speedup 1.0000x reference)
"""Correlation-layer (cost volume) kernel for 8 Trainium2 NeuronCores.

Problem: out[n, 0, h, w, dy*41+dx] = sum_c fm1[n,c,h,w] * fm2p[n,c,h+dy,w+dx]
with fm2p = fm2 zero-padded by 20 on both spatial axes, dy,dx in [0,41).

Sharding: core k handles batch n = k//2 and h-slab [64*(k%2), 64*(k%2)+64).
No cross-core communication: each core's fm2 slab (with a 20-row halo) is
prepared on the host.

Device algorithm (per core, fp16 in / fp32 PSUM / fp16 out):
  - The per-position channel dot product runs on the tensor engine as banded
    matmuls: stationary lhsT = fm1[c, h, wtile] (K=64, M=TILE_M), moving
    rhs = fm2 rows h+dy x padded cols [wtile, wtile+TILE_M+40) -> PSUM gets
    out[w, dy, w'] in *absolute* column coords w' = w+dx.
  - PE array rows 0-63 compute even h, rows 64-127 odd h (fm1/fm2 duplicated
    per half on the host); col position = the w-tile.  Measurement showed
    matmuls execute serially at ~N cycles each (tile_position concurrency
    does not materialize through this stack), so TILE_M=64 minimizes total
    moving columns: (128/M)*(M+40) per (h,dy).
  - PSUM is evacuated (fp32->fp16 cast) by DVE (even h) and ACT (odd h) into
    an SBUF row buffer, then DMA'd out as big contiguous runs.
  - The absolute->relative shear (w' -> dx = w'-w) cannot be done on-chip
    (needs per-partition offsets no engine AP supports); the M+40-wide band
    is written and the host extracts the 41 diagonals with a zero-copy
    as_strided view during the fp32 upcast.
"""

import os
import sys

import numpy as np

for _p in ("/opt/trn_rl_repo",):
    if os.path.isdir(_p) and _p not in sys.path:
        sys.path.append(_p)

# ---- problem constants (hardcoded per contest rules) ----
B, C, H, W = 4, 64, 128, 128
MD = 20                  # max displacement
D = 2 * MD + 1           # 41 displacements per axis
PW = W + 2 * MD          # 168 padded width
HS = H // 2              # 64-row h-slab per core
RS = HS + 2 * MD         # 104 fm2 slab rows (with halo)
NCORES = 8

TILE_M = 64              # w-tile width (PE stationary M); 32 or 64
NT = W // TILE_M         # col tiles
WIN = TILE_M + 2 * MD    # absolute-coord band window per w-tile
FREE = D * WIN           # fp16 values per (h, w) written to HBM
ND_MAX = 512 // WIN      # dy rows per matmul s.t. output fits one PSUM bank


def _dy_chunks():
    out, dy0 = [], 0
    while dy0 < D:
        nd = min(ND_MAX, D - dy0)
        out.append((dy0, nd))
        dy0 += nd
    return out


DY_CHUNKS = _dy_chunks()

_CACHE = {}


def _build_program(io_dtype_name="float16", loop_k=0):
    """Build + compile the single-core SPMD Bass program.

    loop_k > 0 builds a TIMING variant: the compute loop runs loop_k times
    inside a device-side For_i, output goes to Internal DRAM, and only a tiny
    marker tensor is an ExternalOutput, so wall-clock deltas between loop_k
    values measure pure on-device time independent of axon transfers.
    """
    import contextlib

    from concourse import bacc
    import concourse.mybir as mybir
    import concourse.tile as tile

    dt_io = getattr(mybir.dt, io_dtype_name)

    nc = bacc.Bacc("TRN2", target_bir_lowering=False, debug=False)
    fm1_d = nc.dram_tensor("fm1s", [128, HS // 2, W], dt_io, kind="ExternalInput").ap()
    fm2_d = nc.dram_tensor("fm2s", [128, RS, PW], dt_io, kind="ExternalInput").ap()
    out_kind = "Internal" if loop_k else "ExternalOutput"
    out_d = nc.dram_tensor(
        "outs", [HS // 2, 128, 2, FREE], dt_io, kind=out_kind
    ).ap()
    marker_d = None
    if loop_k:
        marker_d = nc.dram_tensor(
            "marker", [1, 8], mybir.dt.float32, kind="ExternalOutput"
        ).ap()

    with tile.TileContext(nc) as tc:
        with (
            tc.tile_pool(name="const", bufs=1) as cpool,
            tc.tile_pool(name="srow", bufs=3) as spool,
            tc.tile_pool(name="psum", bufs=4, space="PSUM") as ppool,
        ):
            fm1_sb = cpool.tile([128, HS // 2, W], dt_io)
            fm2_sb = cpool.tile([128, RS, PW], dt_io)
            nc.sync.dma_start(fm1_sb[:], fm1_d[:])
            nc.sync.dma_start(fm2_sb[:], fm2_d[:])

            loop_cm = tc.For_i(0, loop_k, 1) if loop_k else contextlib.nullcontext()
            with loop_cm:
                for hp in range(HS // 2):
                    S = spool.tile([128, 2, FREE], dt_io, tag="S")
                    for dy0, nd in DY_CHUNKS:
                        # one single-bank PSUM tile per h parity
                        ps = [
                            ppool.tile(
                                [128, 512], mybir.dt.float32,
                                name=f"ps{i}", tag=f"ps{i}",
                            )
                            for i in range(2)
                        ]
                        for c4 in range(NT):
                            for hsub in range(2):
                                rb = 64 * hsub
                                r0 = 2 * hp + hsub + dy0
                                nc.tensor.matmul(
                                    ps[hsub][
                                        TILE_M * c4 : TILE_M * (c4 + 1),
                                        0 : nd * WIN,
                                    ],
                                    fm1_sb[
                                        rb : rb + 64, hp,
                                        TILE_M * c4 : TILE_M * (c4 + 1),
                                    ],
                                    fm2_sb[
                                        rb : rb + 64, r0 : r0 + nd,
                                        TILE_M * c4 : TILE_M * c4 + WIN,
                                    ],
                                    start=True,
                                    stop=True,
                                    tile_position=(rb, TILE_M * c4),
                                )
                        # evacuate PSUM -> SBUF (cast fp32 -> io dtype)
                        for hsub in range(2):
                            copy = (
                                nc.vector.tensor_copy
                                if hsub == 0
                                else nc.scalar.copy
                            )
                            copy(
                                S[:, hsub, dy0 * WIN : (dy0 + nd) * WIN],
                                ps[hsub][:, 0 : nd * WIN],
                            )
                    nc.sync.dma_start(out_d[hp], S[:])

            if loop_k:
                mk = cpool.tile([1, 8], mybir.dt.float32, name="mk")
                nc.vector.memset(mk[:], 1.0)
                nc.sync.dma_start(marker_d[:], mk[:])

    nc.compile()
    return nc


def _get_compiled(io_dtype_name="float16", loop_k=0):
    key = ("prog", io_dtype_name, loop_k)
    if key not in _CACHE:
        _CACHE[key] = _build_program(io_dtype_name, loop_k)
    return _CACHE[key]


def shard_inputs(fm1, fm2, np_dtype=np.float16):
    """Full (4,64,128,128) inputs -> 8 per-core input dicts."""
    fm1 = np.asarray(fm1, dtype=np.float32)
    fm2 = np.asarray(fm2, dtype=np.float32)
    in_maps = []
    for k in range(NCORES):
        n, hbase = k // 2, (k % 2) * HS
        a = fm1[n].astype(np_dtype)                      # (C, H, W)
        slab = a[:, hbase : hbase + HS]                  # (C, 64, W)
        fm1s = np.concatenate([slab[:, 0::2], slab[:, 1::2]], axis=0)
        fm1s = np.ascontiguousarray(fm1s)                # (128, 32, W)

        p = np.zeros((C, H + 2 * MD, PW), dtype=np_dtype)
        p[:, MD : MD + H, MD : MD + W] = fm2[n].astype(np_dtype)
        slab2 = p[:, hbase : hbase + RS]                 # (C, 104, 168)
        fm2s = np.ascontiguousarray(np.concatenate([slab2, slab2], axis=0))
        in_maps.append({"fm1s": fm1s, "fm2s": fm2s})
    return in_maps


def unshard_outputs(results):
    """8 per-core {'outs': (32,128,2,D*WIN)} -> full (4,1,128,128,1681) fp32."""
    out = np.empty((B, 1, H, W, D * D), dtype=np.float32)
    for k in range(NCORES):
        n, hbase = k // 2, (k % 2) * HS
        g = np.asarray(results[k]["outs"])
        a = g.reshape(HS // 2, NT, TILE_M, 2, D, WIN)  # [hp, t, u, hsub, dy, win]
        st = a.strides
        band = np.lib.stride_tricks.as_strided(
            a,
            shape=(HS // 2, 2, NT, TILE_M, D, D),
            strides=(st[0], st[3], st[1], st[2] + st[5], st[4], st[5]),
        )
        out[n, 0, hbase : hbase + HS] = (
            band.astype(np.float32).reshape(HS, W, D * D)
        )
    return out


def run_on_hw(in_maps, io_dtype_name="float16", trace=False, **kw):
    from concourse import bass_utils

    nc = _get_compiled(io_dtype_name)
    res = bass_utils.run_bass_kernel_spmd(
        nc, in_maps, list(range(NCORES)), trace=trace, **kw
    )
    return res


def kernel(feature_map_1, feature_map_2):
    in_maps = shard_inputs(feature_map_1, feature_map_2)
    res = run_on_hw(in_maps)
    return unshard_outputs(res.results)


if __name__ == "__main__":
    inputs = {
        "feature_map_1": np.random.randn(B, C, H, W).astype(np.float32),
        "feature_map_2": np.random.randn(B, C, H, W).astype(np.float32),
    }
    out = kernel(**inputs)
    print("kernel output", out.shape, out.dtype)



# revision 10
# speedup vs baseline: 1.3568x; 1.3568x over previous
"""Correlation-layer (cost volume) kernel for 8 Trainium2 NeuronCores.

Problem: out[n, 0, h, w, dy*41+dx] = sum_c fm1[n,c,h,w] * fm2p[n,c,h+dy,w+dx]
with fm2p = fm2 zero-padded by 20 on both spatial axes, dy,dx in [0,41).

Sharding: core k handles batch n = k//2 and h-slab [64*(k%2), 64*(k%2)+64).
No cross-core communication: each core's fm2 slab (with a 20-row halo) is
prepared on the host.

Device algorithm (per core, fp16 in / fp32 PSUM / fp16 out), v2:
  - Stationary = a 16x8 (h,w)-BLOCK of fm1 (K=64 channels, M=128 = 16*8
    pixels).  One stationary load then serves all 41*41 displacements for
    128 output pixels: the moving stream is fm2 rows [h0, h0+56) x padded
    cols [w0, w0+48), i.e. (16+40)*(8+40) = 2688 columns per load instead
    of the 8528 the per-row band formulation needs.  64 loads total.
  - Moving is streamed in 7 chunks of 8 rows x 48 cols = 384 fp32 columns,
    each into its own PSUM bank; PSUM[p = h_in*8+w_in, r_rel*48 + w_rel] =
    <fm1[:,h,w], fm2[:,h0+r_rel,w0+w_rel]>.
  - PSUM is evacuated (fp32->fp16) into an SBUF band tile S[128, 56, 48]
    split across DVE / ACT / Pool so no single engine bottlenecks.
  - Output DMA per load either writes the full band (TRIM=False) or uses a
    custom 3-dim addr64 access pattern whose outer stride advances 8
    partitions AND one 48-col band row at once (TRIM=True), storing only
    the 41 rows [h_in, h_in+41) each partition actually needs.
  - The w-direction shear (dx = w' - w_in) cannot be expressed in <=3 DMA
    dims; the host extracts the 41 diagonal columns with a zero-copy
    as_strided view during the fp32 upcast.
"""

import os
import sys

import numpy as np

for _p in ("/opt/trn_rl_repo",):
    if os.path.isdir(_p) and _p not in sys.path:
        sys.path.append(_p)

# ---- problem constants (hardcoded per contest rules) ----
B, C, H, W = 4, 64, 128, 128
MD = 20                  # max displacement
D = 2 * MD + 1           # 41 displacements per axis
PW = W + 2 * MD          # 168 padded width
HS = H // 2              # 64-row h-slab per core
RS = HS + 2 * MD         # 104 fm2 slab rows (with halo)
NCORES = 8

NH, NW = 16, 8           # fm1 pixel block per stationary load (NH*NW = 128)
NHB, NWB = HS // NH, W // NW      # 4 x 16 = 64 loads
BR, BC = NH + 2 * MD, NW + 2 * MD  # 56 x 48 band per load
CH = 8                   # moving rows per matmul chunk
NCH = BR // CH           # 7 chunks/load, 8*48 = 384 fp32 <= 512 (1 PSUM bank)

TRIM = True              # row-trimmed output DMA via custom addr64 AP

_CACHE = {}


def _build_program(io_dtype_name="float16", trim=TRIM):
    from concourse import bacc
    import concourse.mybir as mybir
    import concourse.tile as tile

    dt_io = getattr(mybir.dt, io_dtype_name)

    nc = bacc.Bacc("TRN2", target_bir_lowering=False, debug=False)
    # fm1 host-bl blocked: [c, hb, wb, h_in*NW + w_in] so a stationary load is
    # one contiguous 128-wide slice (BIR: stationary AP = single free dim).
    fm1_d = nc.dram_tensor(
        "fm1s", [C, NHB, NWB, NH * NW], dt_io, kind="ExternalInput"
    ).ap()
    fm2_d = nc.dram_tensor("fm2s", [C, RS, PW], dt_io, kind="ExternalInput").ap()
    # trim: partitions 0-63 (h_in 0-7) only need band rows [0,48); partitions
    # 64-127 (h_in 8-15) only rows [8,56).  Two half-DMAs per load store
    # 48x48 per pixel instead of 56x48.
    TR = BR - CH         # 48 trimmed rows per half
    if trim:
        out_shape = [NHB * NWB, 2, 64, TR * BC]
    else:
        out_shape = [NHB * NWB, NH * NW, BR * BC]
    out_d = nc.dram_tensor("outs", out_shape, dt_io, kind="ExternalOutput").ap()

    with tile.TileContext(nc) as tc:
        with (
            tc.tile_pool(name="const", bufs=1) as cpool,
            tc.tile_pool(name="srow", bufs=3) as spool,
            tc.tile_pool(name="psum", bufs=1, space="PSUM") as ppool,
        ):
            fm1_sb = cpool.tile([C, NHB, NWB, NH * NW], dt_io)
            fm2_sb = cpool.tile([C, RS, PW], dt_io)
            nc.sync.dma_start(fm1_sb[:], fm1_d[:])
            nc.sync.dma_start(fm2_sb[:], fm2_d[:])

            for hb in range(NHB):
                for wb in range(NWB):
                    li = hb * NWB + wb
                    S = spool.tile([128, BR, BC], dt_io, tag="S")
                    for j in range(NCH):
                        ps = ppool.tile(
                            [128, CH, BC], mybir.dt.float32,
                            name=f"ps{j}", tag=f"ps{j}",
                        )
                        nc.tensor.matmul(
                            ps[:],
                            fm1_sb[:, hb, wb, :],
                            fm2_sb[:, NH * hb + CH * j : NH * hb + CH * (j + 1),
                                   NW * wb : NW * wb + BC],
                            start=True,
                            stop=True,
                        )
                        # evacuate PSUM -> SBUF (fp32 -> io dtype cast);
                        # GPSIMD can't read PSUM, so split ACT 4/7, DVE 3/7
                        copy = (
                            nc.scalar.copy if j % 2 == 0
                            else nc.vector.tensor_copy
                        )
                        copy(S[:, CH * j : CH * (j + 1), :], ps[:])
                    if trim:
                        nc.sync.dma_start(out_d[li, 0], S[0:64, 0:TR, :])
                        nc.sync.dma_start(out_d[li, 1], S[64:128, CH:BR, :])
                    else:
                        nc.sync.dma_start(out_d[li], S[:])

    nc.compile()
    return nc


def _get_compiled(io_dtype_name="float16", trim=TRIM):
    key = ("prog", io_dtype_name, trim)
    if key not in _CACHE:
        _CACHE[key] = _build_program(io_dtype_name, trim)
    return _CACHE[key]


def shard_inputs(fm1, fm2, np_dtype=np.float16):
    """Full (4,64,128,128) inputs -> 8 per-core input dicts."""
    fm1 = np.asarray(fm1, dtype=np.float32)
    fm2 = np.asarray(fm2, dtype=np.float32)
    in_maps = []
    for k in range(NCORES):
        n, hbase = k // 2, (k % 2) * HS
        slab = fm1[n, :, hbase : hbase + HS].astype(np_dtype)  # (C, 64, 128)
        fm1s = np.ascontiguousarray(
            slab.reshape(C, NHB, NH, NWB, NW)
            .transpose(0, 1, 3, 2, 4)
            .reshape(C, NHB, NWB, NH * NW)
        )
        p = np.zeros((C, H + 2 * MD, PW), dtype=np_dtype)
        p[:, MD : MD + H, MD : MD + W] = fm2[n].astype(np_dtype)
        fm2s = np.ascontiguousarray(p[:, hbase : hbase + RS])  # (C, 104, 168)
        in_maps.append({"fm1s": fm1s, "fm2s": fm2s})
    return in_maps


def unshard_outputs(results, trim=TRIM):
    """8 per-core band outputs -> full (4,1,128,128,1681) fp32."""
    out = np.empty((B, 1, H, W, D * D), dtype=np.float32)
    for k in range(NCORES):
        n, hbase = k // 2, (k % 2) * HS
        g = np.asarray(results[k]["outs"])
        if trim:
            TR = BR - CH
            # [hb, wb, half, h_in', w_in, r - 8*half, w']
            a = g.reshape(NHB, NWB, 2, CH, NW, TR, BC)
            st = a.strides
            band = np.lib.stride_tricks.as_strided(
                a,
                shape=(NHB, NWB, 2, CH, NW, D, D),
                strides=(st[0], st[1], st[2], st[3] + st[5], st[4] + st[6],
                         st[5], st[6]),
            )
            out[n, 0, hbase : hbase + HS] = (
                band.transpose(0, 2, 3, 1, 4, 5, 6)
                .astype(np.float32)
                .reshape(HS, W, D * D)
            )
            continue
        else:
            a = g.reshape(NHB, NWB, NH, NW, BR, BC)
            st = a.strides
            band = np.lib.stride_tricks.as_strided(
                a,
                shape=(NHB, NWB, NH, NW, D, D),
                strides=(st[0], st[1], st[2] + st[4], st[3] + st[5],
                         st[4], st[5]),
            )
        out[n, 0, hbase : hbase + HS] = (
            band.transpose(0, 2, 1, 3, 4, 5)
            .astype(np.float32)
            .reshape(HS, W, D * D)
        )
    return out


def run_on_hw(in_maps, io_dtype_name="float16", trace=False, **kw):
    from concourse import bass_utils

    nc = _get_compiled(io_dtype_name)
    res = bass_utils.run_bass_kernel_spmd(
        nc, in_maps, list(range(NCORES)), trace=trace, **kw
    )
    return res


def kernel(feature_map_1, feature_map_2):
    in_maps = shard_inputs(feature_map_1, feature_map_2)
    res = run_on_hw(in_maps)
    return unshard_outputs(res.results)


if __name__ == "__main__":
    inputs = {
        "feature_map_1": np.random.randn(B, C, H, W).astype(np.float32),
        "feature_map_2": np.random.randn(B, C, H, W).astype(np.float32),
    }
    out = kernel(**inputs)
    print("kernel output", out.shape, out.dtype)
